# revision 1
# baseline (speedup 1.0000x reference)
"""Multi-head attention (B=2, N=2048, C=1024, H=16, D=64) on 8 TRN2 NeuronCores.

Sharding: tensor-parallel over heads. Core i owns heads (2i, 2i+1):
  - qkv weight columns for those heads (Q^T/K^T/V^T computed on device),
  - attention for 4 (batch, head) instances,
  - partial projection y_i = O_i @ W_proj[:, cols_i].T  (row-parallel proj).
Host gathers: y = sum_i y_i + b_proj.

Per-core pipeline (per batch):
  qkv:   Q^T,K^T [128(d,2 heads),2048] and V^T -> PE-transpose -> V_aug [n,130]
         (V columns + a ones column per head, so PV also yields softmax row-sums)
  attn:  per 512-query tile, loop over 16 key tiles:
         S^T[k,h,q] = K^T.T @ Q^T (f32r), exp on ScalarE (PSUM->SBUF, f32r),
         O~aug^T[65,q] += V_aug.T @ P~ (PSUM accumulation, row 64 = sum exp)
  norm:  recip(row 64) -> broadcast over 64 partitions via K=1 matmul -> DVE mult
  proj:  y[q,o] = sum_h O_norm^T_h.T @ WpT_h (PSUM accumulation over heads)

All matmuls run as float32r (1 cycle/row at free-dim>=256, ~tf32 precision);
softmax and accumulations stay float32.
"""
import sys
import types

import numpy as np

B = 2
N = 2048
C = 1024
H = 16
D = 64
SCALE = D ** -0.5
NCORES = 8
HPC = H // NCORES  # heads per core = 2
BN = B * N


def _install_ntff_shim():
    """The image's antenv lacks axon_hooks; provide it so trace=True works."""
    if "antenv.axon_hooks" in sys.modules:
        return
    mod = types.ModuleType("antenv.axon_hooks")
    mod._HOOK = None
    mod.set_axon_ntff_profile_hook = lambda h: setattr(mod, "_HOOK", h)
    mod.get_axon_ntff_profile_hook = lambda: mod._HOOK
    sys.modules["antenv.axon_hooks"] = mod
    if "/root/.axon_site" not in sys.path:
        sys.path.insert(0, "/root/.axon_site")
    try:
        from trn_agent_boot.trn_boot import _ntff_profile_via_ctypes

        mod.set_axon_ntff_profile_hook(
            _ntff_profile_via_ctypes("/opt/axon/libaxon_pjrt.so")
        )
    except Exception:
        pass


_install_ntff_shim()

import concourse.bass as bass  # noqa: E402
import concourse.tile as tile  # noqa: E402
from concourse import bacc, mybir  # noqa: E402
from concourse.bass_utils import run_bass_kernel_spmd  # noqa: E402
from concourse.masks import make_identity  # noqa: E402

F32 = mybir.dt.float32
F32R = mybir.dt.float32r
EXP = mybir.ActivationFunctionType.Exp

_NC_CACHE = {}


def build_nc():
    nc = bacc.Bacc(None, target_bir_lowering=False)

    xT_ext = nc.declare_dram_parameter("xT", [C, BN], F32R, isOutput=False)
    wqT_ext = nc.declare_dram_parameter("wqT", [C, 128], F32R, isOutput=False)
    wkT_ext = nc.declare_dram_parameter("wkT", [C, 128], F32R, isOutput=False)
    wvT_ext = nc.declare_dram_parameter("wvT", [C, 128], F32R, isOutput=False)
    wpT_ext = nc.declare_dram_parameter("wpT", [HPC, D, C], F32R, isOutput=False)
    out_ext = nc.declare_dram_parameter("out", [BN, C], F32, isOutput=True)

    NT = N // 512          # 512-token tiles per batch (4)
    NKT = N // 128         # 128-token key tiles per batch (16)
    CO = C // 128          # contraction chunks (8)

    with tile.TileContext(nc) as tc:
        with (
            tc.tile_pool(name="consts", bufs=1) as consts,
            tc.tile_pool(name="weights", bufs=1) as weights,
            tc.tile_pool(name="xpool", bufs=10) as xpool,
            tc.tile_pool(name="qkv", bufs=2) as qkvp,
            tc.tile_pool(name="work", bufs=3) as work,
            tc.tile_pool(name="small", bufs=2) as small,
            tc.tile_pool(name="onorm", bufs=4) as onormp,
            tc.tile_pool(name="ypool", bufs=3) as ypool,
            tc.tile_pool(name="ps_s", bufs=2, space="PSUM") as ps_s,
            tc.tile_pool(name="ps_o", bufs=2, space="PSUM") as ps_o,
            tc.tile_pool(name="ps_mm", bufs=2, space="PSUM") as ps_mm,
        ):
            ident = consts.tile([128, 128], F32)
            make_identity(nc, ident[:])
            ones = consts.tile([128, 64], F32)
            nc.vector.memset(ones[:], 1.0)

            wq = weights.tile([128, CO, 128], F32R)
            wk = weights.tile([128, CO, 128], F32R)
            wv = weights.tile([128, CO, 128], F32R)
            nc.sync.dma_start(wq[:], wqT_ext.rearrange("(co p) d -> p co d", p=128))
            nc.sync.dma_start(wk[:], wkT_ext.rearrange("(co p) d -> p co d", p=128))
            nc.sync.dma_start(wv[:], wvT_ext.rearrange("(co p) d -> p co d", p=128))
            wp = weights.tile([D, HPC, C], F32R)
            nc.sync.dma_start(wp[:], wpT_ext.rearrange("h p o -> p h o"))

            for b in range(B):
                # ---- load x^T chunks for this batch ----
                xb = []
                for co in range(CO):
                    xc = xpool.tile([128, N], F32R, tag="xchunk")
                    nc.sync.dma_start(
                        xc[:],
                        xT_ext[co * 128:(co + 1) * 128, b * N:(b + 1) * N],
                    )
                    xb.append(xc)

                # ---- qkv projections ----
                QT = qkvp.tile([128, N], F32R, tag="qt")
                KT = qkvp.tile([128, N], F32R, tag="kt")
                VA = qkvp.tile([128, NKT, 2 * (D + 1)], F32R, tag="vaug")
                # ones columns (64 and 129) for the PV row-sum trick
                nc.vector.tensor_copy(VA[:, :, D], ones[:, 0:NKT])
                nc.vector.tensor_copy(VA[:, :, 2 * D + 1], ones[:, 0:NKT])

                for t in range(NT):
                    ts = slice(t * 512, (t + 1) * 512)
                    for dst, w in ((QT, wq), (KT, wk)):
                        ps = ps_mm.tile([128, 512], F32, tag="mm512")
                        for co in range(CO):
                            nc.tensor.matmul(
                                ps[:],
                                lhsT=w[:, co, :],
                                rhs=xb[co][:, ts],
                                start=(co == 0),
                                stop=(co == CO - 1),
                            )
                        nc.vector.tensor_copy(dst[:, ts], ps[:])
                    # V^T tile then PE-transpose into VA
                    psv = ps_mm.tile([128, 512], F32, tag="mm512")
                    for co in range(CO):
                        nc.tensor.matmul(
                            psv[:],
                            lhsT=wv[:, co, :],
                            rhs=xb[co][:, ts],
                            start=(co == 0),
                            stop=(co == CO - 1),
                        )
                    vt = work.tile([128, 512], F32, tag="vt")
                    nc.vector.tensor_copy(vt[:], psv[:])
                    for j in range(4):
                        pst = ps_mm.tile([128, 512], F32, tag="mm512")
                        nc.tensor.transpose(
                            pst[:, 0:128], vt[:, j * 128:(j + 1) * 128], ident[:]
                        )
                        nt = t * 4 + j
                        # cols 0:64 -> head0 slot, 64:128 -> head1 slot
                        nc.vector.tensor_copy(
                            VA[:, nt, 0:D], pst[:, 0:D]
                        )
                        nc.vector.tensor_copy(
                            VA[:, nt, D + 1:2 * D + 1], pst[:, D:2 * D]
                        )

                # ---- attention + projection per 512-query tile ----
                for qt in range(NT):
                    qs = slice(qt * 512, (qt + 1) * 512)
                    O0 = ps_o.tile([D + 1, 512], F32, tag="oacc")
                    O1 = ps_o.tile([D + 1, 512], F32, tag="oacc")
                    for kt in range(NKT):
                        ks = slice(kt * 128, (kt + 1) * 128)
                        S = ps_s.tile([128, 2, 512], F32, tag="s")
                        nc.tensor.matmul(
                            S[:, 0, :], lhsT=KT[0:D, ks], rhs=QT[0:D, qs],
                            start=True, stop=True,
                        )
                        nc.tensor.matmul(
                            S[:, 1, :], lhsT=KT[D:2 * D, ks], rhs=QT[D:2 * D, qs],
                            start=True, stop=True,
                        )
                        P = work.tile([128, 2, 512], F32R, tag="p")
                        nc.scalar.activation(P[:], S[:], EXP)
                        nc.tensor.matmul(
                            O0[:], lhsT=VA[:, kt, 0:D + 1], rhs=P[:, 0, :],
                            start=(kt == 0), stop=(kt == NKT - 1),
                        )
                        nc.tensor.matmul(
                            O1[:], lhsT=VA[:, kt, D + 1:2 * (D + 1)], rhs=P[:, 1, :],
                            start=(kt == 0), stop=(kt == NKT - 1),
                        )
                    # normalize: O_norm^T_h = O~^T_h * (1/rowsum_h) broadcast
                    On = []
                    for h, O in ((0, O0), (1, O1)):
                        rec = small.tile([128, 512], F32, tag="rec")
                        nc.vector.reciprocal(rec[D:D + 1, :], O[D:D + 1, :])
                        bc = ps_mm.tile([128, 512], F32, tag="mm512")
                        nc.tensor.matmul(
                            bc[0:D, :], lhsT=ones[D:D + 1, 0:D],
                            rhs=rec[D:D + 1, :], start=True, stop=True,
                        )
                        bcs = small.tile([D, 512], F32, tag="bcs")
                        nc.vector.tensor_copy(bcs[:], bc[0:D, :])
                        on = onormp.tile([D, 512], F32R, tag="onorm")
                        nc.vector.tensor_mul(on[:], O[0:D, :], bcs[:])
                        On.append(on)
                    # projection: y[q, o] = sum_h On_h.T @ wp_h
                    for sq in range(4):
                        sqs = slice(sq * 128, (sq + 1) * 128)
                        row0 = b * N + qt * 512 + sq * 128
                        for ot in range(2):
                            os_ = slice(ot * 512, (ot + 1) * 512)
                            yps = ps_mm.tile([128, 512], F32, tag="mm512")
                            nc.tensor.matmul(
                                yps[:], lhsT=On[0][:, sqs], rhs=wp[:, 0, os_],
                                start=True, stop=False,
                            )
                            nc.tensor.matmul(
                                yps[:], lhsT=On[1][:, sqs], rhs=wp[:, 1, os_],
                                start=False, stop=True,
                            )
                            ysb = ypool.tile([128, 512], F32, tag="y")
                            nc.vector.tensor_copy(ysb[:], yps[:])
                            nc.sync.dma_start(
                                out_ext[row0:row0 + 128, os_], ysb[:]
                            )
    nc.finalize()
    return nc


def _host_prep(x, W_qkv, W_proj):
    xT = np.ascontiguousarray(x.reshape(BN, C).T).astype(np.float32)
    in_maps = []
    for i in range(NCORES):
        hs = [HPC * i + j for j in range(HPC)]
        wq = np.concatenate([W_qkv[h * D:(h + 1) * D, :] for h in hs], 0)
        wk = np.concatenate([W_qkv[C + h * D:C + (h + 1) * D, :] for h in hs], 0)
        wv = np.concatenate([W_qkv[2 * C + h * D:2 * C + (h + 1) * D, :] for h in hs], 0)
        wqT = np.ascontiguousarray((wq * SCALE).T).astype(np.float32)
        wkT = np.ascontiguousarray(wk.T).astype(np.float32)
        wvT = np.ascontiguousarray(wv.T).astype(np.float32)
        wpT = np.stack(
            [np.ascontiguousarray(W_proj[:, h * D:(h + 1) * D].T) for h in hs], 0
        ).astype(np.float32)
        in_maps.append({"xT": xT, "wqT": wqT, "wkT": wkT, "wvT": wvT, "wpT": wpT})
    return in_maps


def run(x, W_qkv, W_proj, b_proj, trace=False):
    if "nc" not in _NC_CACHE:
        _NC_CACHE["nc"] = build_nc()
    nc = _NC_CACHE["nc"]
    in_maps = _host_prep(np.asarray(x), np.asarray(W_qkv), np.asarray(W_proj))
    res = run_bass_kernel_spmd(
        nc, in_maps, core_ids=list(range(NCORES)), trace=trace
    )
    acc = np.zeros((BN, C), np.float64)
    for i in range(NCORES):
        acc += res.results[i]["out"].astype(np.float64)
    y = (acc + np.asarray(b_proj).astype(np.float64)).astype(np.float32)
    return y.reshape(B, N, C), res


def kernel(x, W_qkv, W_proj, b_proj):
    y, _ = run(x, W_qkv, W_proj, b_proj, trace=False)
    return y


# revision 3
# speedup vs baseline: 1.2685x; 1.2685x over previous
"""Multi-head attention (B=2, N=2048, C=1024, H=16, D=64) on 8 TRN2 NeuronCores.

Sharding: tensor-parallel over heads. Core i owns heads (2i, 2i+1):
  - qkv weight columns for those heads (Q^T/K^T/V^T computed on device),
  - attention for 4 (batch, head) instances,
  - partial projection y_i = O_i @ W_proj[:, cols_i].T  (row-parallel proj).
Host gathers: y = sum_i y_i + b_proj.

Per-core pipeline (per batch):
  qkv:   Q^T,K^T [128(d,2 heads),2048] and V^T -> PE-transpose -> V_aug [n,130]
         (V columns + a ones column per head, so PV also yields softmax row-sums)
  attn:  per 512-query tile, loop over 16 key tiles:
         S^T[k,h,q] = K^T.T @ Q^T (bf16 in, f32 PSUM), exp on ScalarE -> bf16,
         O~aug^T[65,q] += V_aug.T @ P~ (PSUM accumulation, row 64 = sum exp)
  norm:  recip(row 64) -> broadcast over 64 partitions via K=1 matmul -> DVE mult
  proj:  y[q,o] = sum_h O_norm^T_h.T @ WpT_h (PSUM accumulation over heads)

Matmul operands are bf16 (1 cycle/row, FWL weight loads); softmax statistics,
PSUM accumulation and the final output stay float32.
"""
import sys
import types

import numpy as np

B = 2
N = 2048
C = 1024
H = 16
D = 64
SCALE = D ** -0.5
NCORES = 8
HPC = H // NCORES  # heads per core = 2
BN = B * N


def _install_ntff_shim():
    """The image's antenv lacks axon_hooks; provide it so trace=True works."""
    if "antenv.axon_hooks" in sys.modules:
        return
    mod = types.ModuleType("antenv.axon_hooks")
    mod._HOOK = None
    mod.set_axon_ntff_profile_hook = lambda h: setattr(mod, "_HOOK", h)
    mod.get_axon_ntff_profile_hook = lambda: mod._HOOK
    sys.modules["antenv.axon_hooks"] = mod
    if "/root/.axon_site" not in sys.path:
        sys.path.insert(0, "/root/.axon_site")
    try:
        from trn_agent_boot.trn_boot import _ntff_profile_via_ctypes

        mod.set_axon_ntff_profile_hook(
            _ntff_profile_via_ctypes("/opt/axon/libaxon_pjrt.so")
        )
    except Exception:
        pass


_install_ntff_shim()

import ml_dtypes  # noqa: E402

import concourse.bass as bass  # noqa: E402
import concourse.tile as tile  # noqa: E402
from concourse import bacc, mybir  # noqa: E402
from concourse.bass_utils import run_bass_kernel_spmd  # noqa: E402
from concourse.masks import make_identity  # noqa: E402

F32 = mybir.dt.float32
BF16 = mybir.dt.bfloat16
EXP = mybir.ActivationFunctionType.Exp
BFNP = ml_dtypes.bfloat16

_NC_CACHE = {}


def build_nc():
    nc = bacc.Bacc(None, target_bir_lowering=False)

    xT_ext = nc.declare_dram_parameter("xT", [C, BN], BF16, isOutput=False)
    wqT_ext = nc.declare_dram_parameter("wqT", [C, 128], BF16, isOutput=False)
    wkT_ext = nc.declare_dram_parameter("wkT", [C, 128], BF16, isOutput=False)
    wvT_ext = nc.declare_dram_parameter("wvT", [C, 128], BF16, isOutput=False)
    wpT_ext = nc.declare_dram_parameter("wpT", [HPC, D, C], BF16, isOutput=False)
    out_ext = nc.declare_dram_parameter("out", [BN, C], F32, isOutput=True)

    NT = N // 512          # 512-token tiles per batch (4)
    NKT = N // 128         # 128-token key tiles per batch (16)
    CO = C // 128          # contraction chunks (8)

    with tile.TileContext(nc) as tc:
        with (
            tc.tile_pool(name="consts", bufs=1) as consts,
            tc.tile_pool(name="weights", bufs=1) as weights,
            tc.tile_pool(name="xpool", bufs=2 * CO) as xpool,
            tc.tile_pool(name="qkv", bufs=2) as qkvp,
            tc.tile_pool(name="work", bufs=4) as work,
            tc.tile_pool(name="small", bufs=2) as small,
            tc.tile_pool(name="onorm", bufs=4) as onormp,
            tc.tile_pool(name="ypool", bufs=4) as ypool,
            tc.tile_pool(name="ps_s", bufs=2, space="PSUM") as ps_s,
            tc.tile_pool(name="ps_o", bufs=2, space="PSUM") as ps_o,
            tc.tile_pool(name="ps_mm", bufs=2, space="PSUM") as ps_mm,
        ):
            ident = consts.tile([128, 128], BF16)
            make_identity(nc, ident[:])
            ones = consts.tile([128, 64], F32)
            nc.vector.memset(ones[:], 1.0)
            onesb = consts.tile([128, 64], BF16)
            nc.vector.memset(onesb[:], 1.0)

            wq = weights.tile([128, CO, 128], BF16)
            wk = weights.tile([128, CO, 128], BF16)
            wv = weights.tile([128, CO, 128], BF16)
            nc.sync.dma_start(wq[:], wqT_ext.rearrange("(co p) d -> p co d", p=128))
            nc.sync.dma_start(wk[:], wkT_ext.rearrange("(co p) d -> p co d", p=128))
            nc.sync.dma_start(wv[:], wvT_ext.rearrange("(co p) d -> p co d", p=128))
            wp = weights.tile([D, HPC, C], BF16)
            nc.sync.dma_start(wp[:], wpT_ext.rearrange("h p o -> p h o"))

            for b in range(B):
                # ---- load x^T chunks for this batch ----
                xb = []
                for co in range(CO):
                    xc = xpool.tile([128, N], BF16, tag="xchunk")
                    nc.sync.dma_start(
                        xc[:],
                        xT_ext[co * 128:(co + 1) * 128, b * N:(b + 1) * N],
                    )
                    xb.append(xc)

                # ---- qkv projections ----
                QT = qkvp.tile([128, N], BF16, tag="qt")
                KT = qkvp.tile([128, N], BF16, tag="kt")
                VA = qkvp.tile([128, NKT, 2 * (D + 1)], BF16, tag="vaug")
                # ones columns (64 and 129) for the PV row-sum trick
                nc.vector.tensor_copy(VA[:, :, D], onesb[:, 0:NKT])
                nc.vector.tensor_copy(VA[:, :, 2 * D + 1], onesb[:, 0:NKT])

                for t in range(NT):
                    ts = slice(t * 512, (t + 1) * 512)
                    for dst, w in ((QT, wq), (KT, wk)):
                        ps = ps_mm.tile([128, 512], F32, tag="mm512")
                        for co in range(CO):
                            nc.tensor.matmul(
                                ps[:],
                                lhsT=w[:, co, :],
                                rhs=xb[co][:, ts],
                                start=(co == 0),
                                stop=(co == CO - 1),
                            )
                        nc.vector.tensor_copy(dst[:, ts], ps[:])
                    # V^T tile then PE-transpose into VA
                    psv = ps_mm.tile([128, 512], F32, tag="mm512")
                    for co in range(CO):
                        nc.tensor.matmul(
                            psv[:],
                            lhsT=wv[:, co, :],
                            rhs=xb[co][:, ts],
                            start=(co == 0),
                            stop=(co == CO - 1),
                        )
                    vt = work.tile([128, 512], BF16, tag="vt")
                    nc.vector.tensor_copy(vt[:], psv[:])
                    for j in range(4):
                        pst = ps_mm.tile([128, 128], BF16, tag="mm512")
                        nc.tensor.transpose(
                            pst[:], vt[:, j * 128:(j + 1) * 128], ident[:]
                        )
                        nt = t * 4 + j
                        # cols 0:64 -> head0 slot, 64:128 -> head1 slot
                        nc.vector.tensor_copy(VA[:, nt, 0:D], pst[:, 0:D])
                        nc.vector.tensor_copy(
                            VA[:, nt, D + 1:2 * D + 1], pst[:, D:2 * D]
                        )

                # ---- attention + projection per 512-query tile ----
                for qt in range(NT):
                    qs = slice(qt * 512, (qt + 1) * 512)
                    O0 = ps_o.tile([D + 1, 512], F32, tag="oacc")
                    O1 = ps_o.tile([D + 1, 512], F32, tag="oacc")
                    for kt in range(NKT):
                        ks = slice(kt * 128, (kt + 1) * 128)
                        S = ps_s.tile([128, 2, 512], F32, tag="s")
                        nc.tensor.matmul(
                            S[:, 0, :], lhsT=KT[0:D, ks], rhs=QT[0:D, qs],
                            start=True, stop=True, tile_position=(0, 0),
                        )
                        nc.tensor.matmul(
                            S[:, 1, :], lhsT=KT[D:2 * D, ks], rhs=QT[D:2 * D, qs],
                            start=True, stop=True, tile_position=(64, 0),
                        )
                        P = work.tile([128, 2, 512], BF16, tag="p")
                        nc.scalar.activation(P[:], S[:], EXP)
                        nc.tensor.matmul(
                            O0[:], lhsT=VA[:, kt, 0:D + 1], rhs=P[:, 0, :],
                            start=(kt == 0), stop=(kt == NKT - 1),
                        )
                        nc.tensor.matmul(
                            O1[:], lhsT=VA[:, kt, D + 1:2 * (D + 1)], rhs=P[:, 1, :],
                            start=(kt == 0), stop=(kt == NKT - 1),
                        )
                    # normalize: O_norm^T_h = O~^T_h * (1/rowsum_h) broadcast
                    On = []
                    for h, O in ((0, O0), (1, O1)):
                        rec = small.tile([128, 512], F32, tag="rec")
                        nc.vector.reciprocal(rec[D:D + 1, :], O[D:D + 1, :])
                        bc = ps_mm.tile([128, 512], F32, tag="mm512")
                        nc.tensor.matmul(
                            bc[0:D, :], lhsT=ones[D:D + 1, 0:D],
                            rhs=rec[D:D + 1, :], start=True, stop=True,
                        )
                        bcs = small.tile([D, 512], F32, tag="bcs")
                        nc.vector.tensor_copy(bcs[:], bc[0:D, :])
                        on = onormp.tile([D, 512], BF16, tag="onorm")
                        nc.vector.tensor_mul(on[:], O[0:D, :], bcs[:])
                        On.append(on)
                    # projection: y[q, o] = sum_h On_h.T @ wp_h
                    for sq in range(4):
                        sqs = slice(sq * 128, (sq + 1) * 128)
                        row0 = b * N + qt * 512 + sq * 128
                        for ot in range(2):
                            os_ = slice(ot * 512, (ot + 1) * 512)
                            yps = ps_mm.tile([128, 512], F32, tag="mm512")
                            nc.tensor.matmul(
                                yps[:], lhsT=On[0][:, sqs], rhs=wp[:, 0, os_],
                                start=True, stop=False,
                            )
                            nc.tensor.matmul(
                                yps[:], lhsT=On[1][:, sqs], rhs=wp[:, 1, os_],
                                start=False, stop=True,
                            )
                            ysb = ypool.tile([128, 512], F32, tag="y")
                            nc.vector.tensor_copy(ysb[:], yps[:])
                            nc.sync.dma_start(
                                out_ext[row0:row0 + 128, os_], ysb[:]
                            )
    nc.finalize()
    return nc


def _host_prep(x, W_qkv, W_proj):
    xT = np.ascontiguousarray(x.reshape(BN, C).T).astype(BFNP)
    in_maps = []
    for i in range(NCORES):
        hs = [HPC * i + j for j in range(HPC)]
        wq = np.concatenate([W_qkv[h * D:(h + 1) * D, :] for h in hs], 0)
        wk = np.concatenate([W_qkv[C + h * D:C + (h + 1) * D, :] for h in hs], 0)
        wv = np.concatenate([W_qkv[2 * C + h * D:2 * C + (h + 1) * D, :] for h in hs], 0)
        wqT = np.ascontiguousarray((wq * SCALE).T).astype(BFNP)
        wkT = np.ascontiguousarray(wk.T).astype(BFNP)
        wvT = np.ascontiguousarray(wv.T).astype(BFNP)
        wpT = np.stack(
            [np.ascontiguousarray(W_proj[:, h * D:(h + 1) * D].T) for h in hs], 0
        ).astype(BFNP)
        in_maps.append({"xT": xT, "wqT": wqT, "wkT": wkT, "wvT": wvT, "wpT": wpT})
    return in_maps


def run(x, W_qkv, W_proj, b_proj, trace=False):
    if "nc" not in _NC_CACHE:
        _NC_CACHE["nc"] = build_nc()
    nc = _NC_CACHE["nc"]
    in_maps = _host_prep(np.asarray(x), np.asarray(W_qkv), np.asarray(W_proj))
    res = run_bass_kernel_spmd(
        nc, in_maps, core_ids=list(range(NCORES)), trace=trace
    )
    acc = np.zeros((BN, C), np.float64)
    for i in range(NCORES):
        acc += res.results[i]["out"].astype(np.float64)
    y = (acc + np.asarray(b_proj).astype(np.float64)).astype(np.float32)
    return y.reshape(B, N, C), res


def kernel(x, W_qkv, W_proj, b_proj):
    y, _ = run(x, W_qkv, W_proj, b_proj, trace=False)
    return y
